# revision 2
# baseline (speedup 1.0000x reference)
"""Distributed Trainium2 attention kernel v3 (8 NeuronCores).

softmax(Q K^T * scale) V with B=4, H=16, S=2048, D=64, fp32 I/O.
64 (b,h) pairs split across 8 cores, 8 heads/core, heads processed in pairs.

Measured: 218-219us on HW (vs 279-281us baseline), rel err 9.3e-3.

v3 restructure vs baseline:
 - PV matmuls use P^T as the stationary operand (LDWEIGHTS) and V|ones as
   the 65-col moving operand, producing O directly in natural [q, d] layout
   in PSUM, 128-query blocks accumulated over the 16 k-tiles. This deletes
   the whole O^T output stage of the baseline (PSUM->SBUF copies, PE
   identity-transposes, per-column normalize) that cost ~55us of DVE time
   and ~25us of PE time.
 - The exp outputs for a whole q-chunk (16 k-tiles) are kept alive in a big
   [128, 16, 2, 512] fp16 SBUF tile (double-buffered); the PV block chains
   for q-chunk qc drain during qc+1's score/exp slots (8 PV matmuls per
   k-tile slot), so each PSUM O-bank sees its 4 query-block accumulation
   chains strictly sequentially (a start=True bit-clear only precedes
   writes of a block's own columns; other blocks' finished data is
   untouched).
 - Score PSUM tiles get bufs=3 (6 banks, possible now that the O^T psum and
   transpose psum are gone): the score->exp->score WAR chain that capped
   the baseline at ~(exp latency+score)/2 = 840ns/k-tile relaxes to /3.
 - exp: ACT (activation Exp) and DVE (fp16 Schraudolph) split per k-tile
   slot, DVE_EXP_KCS tunable; normalization is a per-128-query-block
   reciprocal+scalar-multiply on DVE straight out of the O PSUM.
"""

import sys

sys.path.insert(0, "/opt/trn_rl_repo")

from collections import deque

import numpy as np

import concourse.bass as bass  # noqa: F401
import concourse.bacc as bacc
import concourse.mybir as mybir
import concourse.tile as tile
from concourse.bass_utils import run_bass_kernel_spmd

B, H, S, D = 4, 16, 2048, 64
N_CORES = 8
HEADS_PER_CORE = (B * H) // N_CORES  # 8

F32 = mybir.dt.float32
F16 = mybir.dt.float16
I32 = mybir.dt.int32
I16 = mybir.dt.int16

QW = 512  # q chunk width (one PSUM bank of fp32 per head-score)
PVW = 65  # PV moving width: 64 d + 1 rowsum (ones column of V)

# k-tile slots whose exp runs on DVE (fp16 Schraudolph) instead of ACT.
# alternating per-qc: 7 DVE slots on even qc, 6 on odd (balance point ~6.5)
DVE_EXP_KCS7 = frozenset({1, 3, 6, 8, 11, 13, 15})
DVE_EXP_KCS6 = frozenset({1, 4, 6, 9, 12, 14})
SCHRAUDOLPH16_A = 1477.3197218702985  # 2^10 / ln 2
SCHRAUDOLPH16_B = 15300.6240234375    # 15*2^10 - 486408/8192


def build_attention_nc(softmax_scale: float, n_heads: int = HEADS_PER_CORE,
                       s: int = S, d: int = D):
    assert n_heads % 2 == 0 and s % 128 == 0 and d == 64
    n_kt = s // 128          # 16 k-tiles
    n_qc = s // QW           # 4 q chunks
    n_qb = QW // 128         # 4 query blocks per chunk
    n_pairs = n_heads // 2

    nc = bacc.Bacc("TRN2", target_bir_lowering=False, debug=False,
                   num_devices=N_CORES)
    q = nc.dram_tensor("q", [n_heads, s, d], F32, kind="ExternalInput").ap()
    k = nc.dram_tensor("k", [n_heads, s, d], F32, kind="ExternalInput").ap()
    v = nc.dram_tensor("v", [n_heads, s, d], F32, kind="ExternalInput").ap()
    ident = nc.dram_tensor("ident", [128, 128], F16, kind="ExternalInput").ap()
    o = nc.dram_tensor("out", [n_heads, s, d], F32, kind="ExternalOutput").ap()

    with tile.TileContext(nc) as tc:
        with (
            tc.tile_pool(name="const", bufs=1) as const_pool,
            tc.tile_pool(name="stage", bufs=2) as stage_pool,
            tc.tile_pool(name="tposed", bufs=2) as t_pool,
            tc.tile_pool(name="ptp", bufs=2) as pt_pool,
            tc.tile_pool(name="outs", bufs=2) as o_pool,
            tc.tile_pool(name="drb", bufs=2, space="DRAM") as dr_pool,
            tc.tile_pool(name="scps", bufs=3, space="PSUM") as sc_pool,
            tc.tile_pool(name="ops", bufs=1, space="PSUM") as ops_pool,
        ):
            zbias = const_pool.tile([128, 1], F32, tag="zbias", name="zbias")
            nc.vector.memset(zbias[:], 0.0)
            idsb = const_pool.tile([128, 128], F16, tag="idsb", name="idsb")
            nc.sync.dma_start(out=idsb[:], in_=ident)
            # Preload the exp table set (~2.7us) while the input DMAs run,
            # so the first real exp doesn't pay it.
            tdum = const_pool.tile([128, 1], F16, tag="tdum", name="tdum")
            nc.scalar.activation(tdum[:], zbias[:],
                                 mybir.ActivationFunctionType.Exp,
                                 bias=zbias[:, 0:1], scale=1.0)

            # HAM warm-up into the (otherwise unused yet) O psum bank;
            # sources a memset tile so it starts before the idsb DMA lands.
            wsrc = const_pool.tile([64, 64], F16, tag="wsrc", name="wsrc")
            nc.vector.memset(wsrc[:], 0.125)
            warm_t = ops_pool.tile([128, n_qb, PVW], F32, tag="O0",
                                   name="warm_t")
            for _ in range(24):
                nc.tensor.matmul(warm_t[0:64, 0, 0:64], lhsT=wsrc[:],
                                 rhs=wsrc[:], start=True, stop=True)
            warm_sb = const_pool.tile([64, 1], F16, tag="warmsb",
                                      name="warm_sb")
            nc.vector.tensor_copy(warm_sb[:], warm_t[0:64, 0, 0:1])

            # fused normalize: one tensor_tensor against a stride-0
            # broadcast of the reciprocal (production pattern, see
            # concourse/kernels/tile_matmul.py)
            USE_TT_BCAST = [True]

            # Work queue: ('mm', fn) = one 8-matmul PV chunk; ('misc', fn) =
            # extraction / store / startup-transpose work.
            pending = deque()
            mm_pending = [0]

            def drain(slots=1):
                mmb, miscb = slots, 2
                while pending and mmb > 0:
                    if pending[0][0] == 'misc' and miscb <= 0:
                        break
                    kind, fn = pending.popleft()
                    fn()
                    if kind == 'mm':
                        mm_pending[0] -= 1
                        mmb -= 1
                    else:
                        miscb -= 1

            def push_mm(fn):
                pending.append(('mm', fn))
                mm_pending[0] += 1

            n_lc = s // 512
            for p in range(n_pairs):
                # ---- loads (same scheme as baseline) ----
                va = stage_pool.tile([128, n_kt, 2, PVW], F16, tag="va",
                                     name="va")
                qs = stage_pool.tile([128, n_kt, 2, d], F16, tag="qs", name="qs")
                ks = stage_pool.tile([128, n_kt, 2, d], F16, tag="ks", name="ks")
                bq = dr_pool.tile([s, 128], F16, tag="bq", name="bq")
                bk = dr_pool.tile([s, 128], F16, tag="bk", name="bk")
                qT = t_pool.tile([128, s], F16, tag="qT", name="qT")
                kT = t_pool.tile([128, s], F16, tag="kT", name="kT")
                nc.vector.memset(va[:, :, :, d:d + 1], 1.0)  # rowsum ones

                tensors = {"q": (q, qs, bq, qT), "k": (k, ks, bk, kT)}

                def cast_chunk(tname, r0, r1, p=p, tensors=tensors):
                    src, stg, _, _ = tensors[tname]
                    csl = slice(r0 // 128, r1 // 128)
                    for hh in range(2):
                        nc.gpsimd.dma_start(
                            out=stg[:, csl, hh, :],
                            in_=src[2 * p + hh][r0:r1].rearrange(
                                "(c p) d -> p c d", p=128))

                def load_chunk(tname, r0, r1, tensors=tensors,
                               bounce_q=None):
                    cast_chunk(tname, r0, r1)
                    _, stg, bnc, tT = tensors[tname]
                    csl = slice(r0 // 128, r1 // 128)
                    bq_eng = bounce_q if bounce_q is not None else nc.sync
                    bq_eng.dma_start(
                        out=bnc[r0:r1].rearrange("(c p) e -> p c e", p=128),
                        in_=stg[:, csl].rearrange("p c h d -> p c (h d)"))
                    nc.sync.dma_start(
                        out=tT[:, r0:r1], in_=bnc[r0:r1], transpose=True)

                def head_tp_unit(tname, j0, hh, tph2, tensors=tensors):
                    # PE identity-transpose of four [128, 64] staging blocks
                    # into the O0 psum bank (viewed as fp16), then one copy
                    # into qT/kT.
                    _, stg, _, tT = tensors[tname]

                    def emit():
                        psl = slice(hh * 64, (hh + 1) * 64)
                        for c in range(4):
                            nc.tensor.transpose(
                                tph2[psl, c, 0:128], stg[:, j0 + c, hh, :],
                                idsb[:], tile_position=(0, hh * 64))
                        nc.vector.tensor_copy(
                            tT[psl, j0 * 128:(j0 + 4) * 128],
                            tph2[psl, :, 0:128])
                    return emit

                def load_v():
                    for hh in range(2):
                        nc.gpsimd.dma_start(
                            out=va[:, :, hh, 0:d],
                            in_=v[2 * p + hh].rearrange(
                                "(c p) d -> p c d", p=128))

                if p == 0:
                    # Startup: PE-transpose the first two chunks (engines
                    # otherwise idle); the O0 psum bank doubles as the
                    # transpose staging area (bit-clears / junk are
                    # overwritten by the first real PV block writes).
                    # V rides the sync queue as raw fp32 with a DVE downcast
                    # (the gpsimd SWDGE descriptor-gen budget, ~1.4us per
                    # cast DMA, is the startup critical path).
                    tph_t = ops_pool.tile([128, n_qb, PVW], F32, tag="O0",
                                          name="tph_t")
                    tph2 = tph_t[:].bitcast(F16)
                    c0 = min(512, s)
                    cast_chunk("k", 0, c0)
                    qs32 = stage_pool.tile([128, c0 // 128, 2, d], F32,
                                           tag="qs32", name="qs32", bufs=1)
                    for hh in range(2):
                        nc.sync.dma_start(
                            out=qs32[:, :, hh, :],
                            in_=q[2 * p + hh][0:c0].rearrange(
                                "(c p) d -> p c d", p=128))
                    for hh in range(2):
                        nc.vector.tensor_copy(qs[:, 0:c0 // 128, hh, :],
                                              qs32[:, :, hh, :])
                    v32 = stage_pool.tile([128, n_kt, 2, d], F32,
                                          tag="v32", name="v32", bufs=1)
                    for hh in range(2):
                        nc.sync.dma_start(
                            out=v32[:, :, hh, :],
                            in_=v[2 * p + hh].rearrange(
                                "(c p) d -> p c d", p=128))
                    for hh in range(2):
                        nc.vector.tensor_copy(va[:, :, hh, 0:d],
                                              v32[:, :, hh, :])
                    # each cast costs ~1us of serial SWDGE descriptor-gen
                    # (plus ~6us of transfer lag), so gpsimd order IS
                    # arrival order: K chunk casts interleave ahead of the
                    # later-needed Q chunks.
                    if n_lc > 1:
                        cast_chunk("k", 512, 1024)
                        cast_chunk("q", 512, 1024)
                    for lc in range(2, n_lc):
                        load_chunk("k", lc * 512, (lc + 1) * 512)
                    for lc in range(2, n_lc):
                        load_chunk("q", lc * 512, (lc + 1) * 512)
                    for tname in ("k", "q"):
                        for hh in range(2):
                            head_tp_unit(tname, 0, hh, tph2)()
                    if n_lc > 1:
                        for tname in ("k", "q"):
                            for hh in range(2):
                                pending.append(
                                    ('misc', head_tp_unit(tname, 4, hh, tph2)))
                else:
                    load_chunk("k", 0, s)
                    load_v()
                    load_chunk("q", 0, s)

                def emit_score(qc, kc, kT=kT, qT=qT):
                    sps = sc_pool.tile([128, 2, QW], F32, tag="sps",
                                       name="sps")
                    ksl = slice(kc * 128, (kc + 1) * 128)
                    qsl = slice(qc * QW, (qc + 1) * QW)
                    for hh in range(2):
                        psl = slice(hh * 64, (hh + 1) * 64)
                        nc.tensor.matmul(
                            sps[:, hh, :],
                            lhsT=kT[psl, ksl],
                            rhs=qT[psl, qsl],
                            start=True, stop=True)
                    return sps

                sps_carry = None
                for qc in range(n_qc):
                    pta = pt_pool.tile([128, n_kt, 2, QW], F16, tag="pta",
                                       name="pta")
                    Ops = [ops_pool.tile([128, n_qb, PVW], F32, tag=f"O{hh}",
                                         name=f"Ops{hh}") for hh in range(2)]

                    def make_pv(Ops_hh, pta=pta, va=va):
                        def gen(hh, j, half):
                            def emit():
                                for kk in range(half * (n_kt // 2),
                                                (half + 1) * (n_kt // 2)):
                                    nc.tensor.matmul(
                                        Ops_hh[:, j, :],
                                        lhsT=pta[:, kk, hh,
                                                 j * 128:(j + 1) * 128],
                                        rhs=va[:, kk, hh, :],
                                        start=(kk == 0),
                                        stop=(kk == n_kt - 1))
                            return emit
                        return gen

                    gens = [make_pv(Ops[hh]) for hh in range(2)]
                    blocks = [(hh, j) for hh in range(2) for j in range(n_qb)]
                    last_qc = (p == n_pairs - 1 and qc == n_qc - 1)
                    for kc in range(n_kt):
                        sps = sps_carry if sps_carry is not None \
                            else emit_score(qc, kc)
                        if kc + 1 < n_kt:
                            sps_carry = emit_score(qc, kc + 1)
                        elif qc + 1 < n_qc:
                            sps_carry = emit_score(qc + 1, 0)
                        else:
                            sps_carry = None
                        if kc in DVE_EXP_KCS7:
                            nc.vector.tensor_scalar(
                                pta[:, kc].bitcast(I16), sps[:],
                                float(softmax_scale) * SCHRAUDOLPH16_A,
                                SCHRAUDOLPH16_B,
                                op0=mybir.AluOpType.mult,
                                op1=mybir.AluOpType.add)
                        else:
                            nc.scalar.activation(
                                pta[:, kc], sps[:],
                                mybir.ActivationFunctionType.Exp,
                                bias=zbias[:, 0:1],
                                scale=float(softmax_scale))
                        drain(2 if mm_pending[0] > 16 else 1)

                    # ---- enqueue PV + extraction for this qc ----
                    # NOTE: a block's two halves MUST stay adjacent — any
                    # other start=True on the same bank between them clears
                    # the block's has_written bits and its half-0 sum is
                    # overwritten instead of accumulated.
                    for hh in range(2):
                        gen = gens[hh]
                        for j in range(n_qb):
                            for half in (0, 1):
                                push_mm(gen(hh, j, half))

                        def make_extract(Ops_hh=Ops[hh], hh=hh, qc=qc, p=p):
                            rec = o_pool.tile([128, n_qb, 1], F32,
                                              tag=f"rec{hh}", name=f"rec{hh}",
                                              bufs=3)
                            ofin = o_pool.tile([128, n_qb, d], F32,
                                               tag=f"ofin{hh}",
                                               name=f"ofin{hh}", bufs=3)

                            def rec_emit():
                                nc.vector.reciprocal(
                                    rec[:], Ops_hh[:, :, d:d + 1])

                            def mul_emit(j0):
                                def emit():
                                    for j in (j0, j0 + 1):
                                        nc.vector.tensor_scalar_mul(
                                            ofin[:, j, :], Ops_hh[:, j, 0:d],
                                            rec[:, j, :])
                                return emit

                            def mul_tt_emit():
                                nc.vector.tensor_tensor(
                                    ofin[:], Ops_hh[:, :, 0:d],
                                    rec[:, :, 0:1].to_broadcast(
                                        [128, n_qb, d]),
                                    op=mybir.AluOpType.mult)

                            def store_emit():
                                nc.sync.dma_start(
                                    out=o[2 * p + hh][qc * QW:(qc + 1) * QW]
                                    .rearrange("(c p) d -> p c d", p=128),
                                    in_=ofin[:])
                            if USE_TT_BCAST[0]:
                                return [rec_emit, mul_tt_emit, store_emit]
                            return [rec_emit, mul_emit(0), mul_emit(2),
                                    store_emit]

                        for fn in make_extract():
                            pending.append(('misc', fn))

            while pending:
                kind, fn = pending.popleft()
                fn()

    nc.compile()
    return nc


def kernel(Q, K, V, is_causal, softmax_scale):
    del is_causal  # documented no-op in the reference
    Q = np.asarray(Q)
    K = np.asarray(K)
    V = np.asarray(V)
    b, h, s, d = Q.shape
    heads = b * h
    hpc = heads // N_CORES

    nc = build_attention_nc(float(softmax_scale), n_heads=hpc, s=s, d=d)

    Qf = np.ascontiguousarray(Q.reshape(heads, s, d), dtype=np.float32)
    Kf = np.ascontiguousarray(K.reshape(heads, s, d), dtype=np.float32)
    Vf = np.ascontiguousarray(V.reshape(heads, s, d), dtype=np.float32)
    ident = np.eye(128, dtype=np.float16)
    in_maps = [
        {
            "q": Qf[c * hpc:(c + 1) * hpc],
            "k": Kf[c * hpc:(c + 1) * hpc],
            "v": Vf[c * hpc:(c + 1) * hpc],
            "ident": ident,
        }
        for c in range(N_CORES)
    ]
    res = run_bass_kernel_spmd(nc, in_maps, list(range(N_CORES)))
    global LAST_RESULT
    LAST_RESULT = res
    out = np.concatenate([res.results[c]["out"] for c in range(N_CORES)], axis=0)
    return out.reshape(b, h, s, d).astype(np.float32)


LAST_RESULT = None
